# revision 17
# baseline (speedup 1.0000x reference)
"""EquivariantProjectorViaSchur — TRN2 Bass kernel (8 NeuronCores, SPMD).

Math (per 64x64 channel block B of W):
    V   = U_y^T B U_x
    P   = A o V + symmetrize(rot 2x2 sub-blocks)
    out = U_y P U_x^T
Pipeline (per core, W shard = 12 c_in block-columns):
    A:  T1T = (U_y^T B)^T per block, via W-stationary f32r matmuls
        (256-wide moving keeps f32r at 1 cyc/col; half the output is
        discarded).  Evac casts to bf16.
    B:  Z[:, o in g] = XG_g @ T1T[:, o in g] (+ sigma terms via +-XJ
        matmuls reading pair-swapped T1T slices) — all bf16, f32 psum.
        The o axis is host-permuted group-major with rot groups split
        [evens | odds] so the sigma matmuls read/write contiguous-ish
        slices and no T1s tensor is ever materialized.
    C:  PE transposes in bf16 (out dtype == in dtype -> bf16 PSUM, so
        the evac runs at the DVE 2x 16-bit rate).
    D:  out = kron(I2, U_y[:,perm]^T)^T @ ZT, bf16 moving, f32 psum,
        evac converts to bf16 for the store.
Evacuations are merged into 2-bank PSUM spans (512-2048 elements per
copy) to amortize the ACT/DVE fixed costs, and split greedily between
the two engines by a cost model.
Sharding: c_in block-columns — core i owns W[:, i*768:(i+1)*768]; the
tiny constant matrices are replicated (precomputed host-side).
"""
import contextlib
import time

import numpy as np
import ml_dtypes

import concourse.bass as bass
import concourse.tile as tile
import concourse.mybir as mybir
from concourse.tile import ScopedClock

F32 = mybir.dt.float32
F32R = mybir.dt.float32r
BF16 = mybir.dt.bfloat16

O = 64
NSTR_CH = 24          # 128-row stripes per b-chunk
NCH = 2               # b chunks of 24 stripes (48 blocks)
NQ = 3                # c quads (4 c-blocks = 256 cols each)
NCORE = 8
CSH = 768             # columns per core shard
NMAT_B = 28           # bf16 const mats: ident, LS4, 10 XG, 8 XJ+, 8 XJ-


# ---------------------------------------------------------------------------
# workarounds for this toolchain
# ---------------------------------------------------------------------------
def _patched_drain_and_barrier(self, tick_clock, wait_clock):
    # this walrus build rejects >1 sem-wait on a Drain: split the tail waits
    drain_inst = self.nc.sync.drain()
    wait_clock.add_sem_waits(drain_inst.ins,
                             ScopedClock({None: tick_clock.global_clock}))
    si = drain_inst.ins.sync_info
    waits = list(si.on_wait) if si is not None else []
    if len(waits) > 1:
        drain_inst.ins.sync_info = mybir.SyncInfo(
            on_wait=waits[:1], on_update=list(si.on_update))
        for i in range(1, len(waits)):
            d2 = self.nc.sync.drain()
            d2.ins.sync_info = mybir.SyncInfo(on_wait=[waits[i]], on_update=[])
    self.nc.all_engine_barrier()
    assert self.sems is not None
    popped = self.nc._tile_sem_poison_stack.pop()
    assert popped is self._sem_poison
    self.nc.clear_and_free_semaphores(list(self.sems.allocated().values()))
    self.nc.all_engine_barrier()


tile.TileContext._drain_and_barrier = _patched_drain_and_barrier


def cap_sync_waits(nc):
    """walrus codegen allows only 1 sem-wait per instruction struct here;
    carry the excess on NoOps inserted just before (same engine/point)."""
    for f in nc.m.functions:
        for blk in f.blocks:
            insts = list(blk.instructions)
            out = []
            ctr = 0
            for ins in insts:
                si = ins.sync_info
                waits = list(si.on_wait) if si is not None else []
                if len(waits) > 1:
                    for i in range(len(waits) - 1):
                        n = mybir.InstNoOp(name=f"{ins.name}_w{ctr}",
                                           ins=[], outs=[])
                        ctr += 1
                        n.engine = ins.engine
                        n.sync_info = mybir.SyncInfo(on_wait=[waits[i]],
                                                     on_update=[])
                        out.append(n)
                    ins.sync_info = mybir.SyncInfo(
                        on_wait=waits[-1:], on_update=list(si.on_update))
                out.append(ins)
            blk.instructions = out


_LDW_PATCHED = False


def enable_ldw_opt():
    """--enable-ldw-opt=true: skip redundant LDWEIGHTS for runs of matmuls
    sharing the same stationary operand."""
    global _LDW_PATCHED
    if _LDW_PATCHED:
        return
    _LDW_PATCHED = True
    import concourse.bass_utils as bu
    orig = bu.bir_verify_and_optimise

    def patched(tmpdir, inp="bir.json", outp="file.neff", arch=None, *,
                dve_root=None):
        real_run = bu.run_command

        def run_hook(argv, **kw):
            argv = ["--enable-ldw-opt=true" if a == "--enable-ldw-opt=false"
                    else a for a in argv]
            return real_run(argv, **kw)
        bu.run_command = run_hook
        try:
            return orig(tmpdir, inp, outp, arch, dve_root=dve_root)
        finally:
            bu.run_command = real_run
    bu.bir_verify_and_optimise = patched


# ---------------------------------------------------------------------------
# host-side precompute of the replicated constants
# ---------------------------------------------------------------------------
def host_precompute(U_y, U_x, mask, block_rows, block_cols):
    rows = np.asarray(block_rows)
    mask = np.asarray(mask)
    U_y64 = np.asarray(U_y, np.float64)
    U_x64 = np.asarray(U_x, np.float64)
    r_rot = set(int(x) for x in rows.tolist())
    nqd = len(rows) // 4
    cols = np.asarray(block_cols)
    for t in range(nqd):
        r = rows[4 * t:4 * t + 4]
        c = cols[4 * t:4 * t + 4]
        assert mask[r, c].all()
        assert r[0] == r[1] and r[2] == r[3] and r[2] == r[0] + 1 \
            and r[0] % 2 == 0
        assert c[0] == c[2] and c[1] == c[3] and c[1] == c[0] + 1 \
            and c[0] % 2 == 0
    groups, seen = [], np.zeros(O, bool)
    for k in range(O):
        if seen[k]:
            continue
        mem = np.where(mask[k] > 0)[0]
        assert (mask[np.ix_(mem, mem)] > 0).all()
        for m in mem:
            seen[m] = True
        groups.append([int(x) for x in mem])
    pi = np.where(np.arange(O) % 2 == 0, 1.0, -1.0)
    eye2 = np.eye(2)

    # o-permutation: rot groups first as [evens | odds], then diag groups
    perm = []
    ginfo = []     # (is_R, slot_start, size, natural members in slot order)
    for mem in groups:
        if int(mem[0]) in r_rot:
            assert len(mem) == (mem[-1] - mem[0] + 1), "rot group gap"
            ev, od = mem[0::2], mem[1::2]
            ginfo.append(dict(is_R=True, st=len(perm), sz=len(mem)))
            perm += list(ev) + list(od)
    for mem in groups:
        if int(mem[0]) not in r_rot:
            st = int(mem[0])
            assert all(int(m) == st + 2 * i for i, m in enumerate(mem)), \
                "diag group not stride-2"
            ginfo.append(dict(is_R=False, st=len(perm), sz=len(mem)))
            perm += list(mem)
    perm = np.asarray(perm)
    assert len(perm) == O

    # f32r const: [kron(I2, U_y[:, perm]) | same] — 256-wide moving for
    # phase A (right half of each product is discarded; >=256 keeps f32r
    # at 1 cyc/col)
    LY = np.kron(eye2, U_y64[:, perm]).astype(np.float32)
    cstr = np.ascontiguousarray(np.concatenate([LY, LY], axis=1))

    # bf16 consts: ident, LS4, XG x10, then per rot group XJ+, XJ-
    matsb = [np.eye(128), np.kron(eye2, U_y64[:, perm].T)]
    for g in ginfo:
        mem = perm[g['st']:g['st'] + g['sz']]
        s = 0.5 if g['is_R'] else 1.0
        a = np.zeros(O)
        a[mem] = 1.0
        XG = s * (U_x64 @ np.diag(a) @ U_x64.T)
        g['gi'] = len(matsb)
        matsb.append(np.kron(eye2, XG))
    for g in ginfo:
        if not g['is_R']:
            continue
        mem = perm[g['st']:g['st'] + g['sz']]
        XJ = np.zeros((O, O))
        for k in mem:
            XJ += 0.5 * pi[k] * np.outer(U_x64[:, k], U_x64[:, k ^ 1])
        g['jp'] = len(matsb)
        matsb.append(np.kron(eye2, XJ.T))
        g['jn'] = len(matsb)
        matsb.append(np.kron(eye2, -XJ.T))
    assert len(matsb) == NMAT_B
    cstb = np.ascontiguousarray(
        np.concatenate(matsb, axis=1).astype(ml_dtypes.bfloat16))
    gkey = tuple((g['is_R'], g['st'], g['sz']) for g in ginfo)
    return cstr, cstb, ginfo, gkey


class _EvacBalancer:
    """Greedy ACT/DVE assignment for PSUM->SBUF copies.

    ACT ACTIVATE: (N + 352)/1.2 ns, dtype-independent.
    DVE from PSUM: fp32 1x -> (N + 151)/0.96; bf16 2x_1P -> (N/2 + 151)/0.96.
    """
    def __init__(self, nc):
        self.nc = nc
        self.t_act = 0.0
        self.t_dve = 0.0

    def copy(self, dst, src, bf16_src=False):
        n = src.free_size()
        c_act = (352.0 + n) / 1.2
        c_dve = (151.0 + (n / 2.0 if bf16_src else float(n))) / 0.96
        if self.t_act + c_act <= self.t_dve + c_dve:
            self.t_act += c_act
            return self.nc.scalar.copy(dst, src)
        else:
            self.t_dve += c_dve
            return self.nc.vector.tensor_copy(dst, src)


# ---------------------------------------------------------------------------
# device kernel (one program, SPMD over 8 cores)
# ---------------------------------------------------------------------------
def build_kernel(ginfo):
    nc = bass.Bass("TRN2", target_bir_lowering=False, debug=False,
                   num_devices=1)
    w = nc.dram_tensor("w", [6144, CSH], F32R, kind="ExternalInput").ap()
    cstr = nc.dram_tensor("cstr", [128, 256], F32R,
                          kind="ExternalInput").ap()
    cstb = nc.dram_tensor("cstb", [128, NMAT_B * 128], BF16,
                          kind="ExternalInput").ap()
    out = nc.dram_tensor("out", [NQ, 6, 128, 2048], BF16,
                         kind="ExternalOutput").ap()

    with tile.TileContext(nc) as tc:
        ctx = contextlib.ExitStack()
        with ctx:
            ev = _EvacBalancer(nc)
            csr_p = ctx.enter_context(tc.tile_pool(name="csr", bufs=1))
            csb_p = ctx.enter_context(tc.tile_pool(name="csb", bufs=1))
            wch_p = ctx.enter_context(tc.tile_pool(name="wch", bufs=17))
            t1T_p = ctx.enter_context(tc.tile_pool(name="t1T", bufs=1))
            z_p = ctx.enter_context(tc.tile_pool(name="z", bufs=2))
            zt_p = ctx.enter_context(tc.tile_pool(name="zt", bufs=2))
            osb_p = ctx.enter_context(tc.tile_pool(name="osb", bufs=4))
            # one shared PSUM pool: 4 bufs x 2 banks = all 8 banks
            ps_p = ctx.enter_context(
                tc.tile_pool(name="ps", bufs=4, space="PSUM"))

            csr = csr_p.tile([128, 256], F32R)
            nc.sync.dma_start(csr[:], cstr[:])
            csb = csb_p.tile([128, NMAT_B * 128], BF16)

            def cmat(i):
                return csb[:, i * 128:(i + 1) * 128]

            ident, LS4 = cmat(0), cmat(1)
            LY2 = csr[:]

            zsb_u = {}
            zt_u = {}

            def emit_A(ch, t1Tq):
                for sg in range(0, NSTR_CH, 4):
                    grp = []
                    for k4 in range(4):
                        s = sg + k4
                        t = wch_p.tile([128, CSH], F32R, tag="w",
                                       name=f"w_{ch}_{s}")
                        r0 = (ch * NSTR_CH + s) * 128
                        nc.sync.dma_start(t[:], w[r0:r0 + 128, :])
                        grp.append(t)
                    if ch == 0 and sg == 0:
                        # bf16 consts aren't needed until phase B; keep
                        # them behind the first W stripes in the queue
                        nc.sync.dma_start(csb[:], cstb[:])
                    for q in range(NQ):
                        for cp in range(2):
                            pa = ps_p.tile([128, 1024], F32, tag="ps",
                                           name="pa")
                            for m in range(4):
                                lhsT = grp[m][:, (q * 2 + cp) * 128:
                                              (q * 2 + cp + 1) * 128]
                                nc.tensor.matmul(
                                    pa[:, m * 256:(m + 1) * 256],
                                    lhsT, LY2)
                            src = pa[:].rearrange(
                                "p (m c) -> p m c", m=4)[:, :, 0:128]
                            ev.copy(
                                t1Tq[q][cp][:, sg * 128:(sg + 4) * 128],
                                src)

            def emit_B(u, t1T):
                tvs = [t1T[cp][:].rearrange("p (b o) -> p b o", o=64)
                       for cp in range(2)]
                zsb = z_p.tile([128, 2 * NSTR_CH * 128], BF16,
                               tag=f"z{u % 2}", name=f"z_{u}")
                zsb_u[u] = zsb
                zv = zsb[:].rearrange("p (c b o) -> p c b o", c=2, o=64)
                # process groups in pairs with two psum tiles in flight so
                # each group's LDW/evac overlaps the partner's matmuls
                rot = [g for g in ginfo if g['is_R']]
                diag = [g for g in ginfo if not g['is_R']]
                pairs = list(zip(rot[0::2], rot[1::2]))
                for ga, gb in pairs:
                    zps = []
                    for g in (ga, gb):
                        st, sz = g['st'], g['sz']
                        zp = ps_p.tile([128, 1024], F32, tag="ps",
                                       name="zp")
                        zps.append(zp)
                        for cp in range(2):
                            nc.tensor.matmul(
                                zp[:, cp * 512:cp * 512 + 288],
                                cmat(g['gi']),
                                tvs[cp][:, :, st:st + sz],
                                start=True, stop=False)
                    for g, zp in zip((ga, gb), zps):
                        st, sz = g['st'], g['sz']
                        h = sz // 2
                        for cp in range(2):
                            zpv = zp[:, cp * 512:cp * 512 + 288] \
                                .rearrange("p (b o) -> p b o", o=6)
                            nc.tensor.matmul(
                                zpv[:, :, 0:h], cmat(g['jp']),
                                tvs[cp][:, :, st + h:st + sz],
                                start=False, stop=False)
                            nc.tensor.matmul(
                                zpv[:, :, h:sz], cmat(g['jn']),
                                tvs[cp][:, :, st:st + h],
                                start=False, stop=True)
                    for g, zp in zip((ga, gb), zps):
                        st, sz = g['st'], g['sz']
                        src = zp[:].rearrange(
                            "p (c x) -> p c x", c=2)[:, :, 0:288]
                        ev.copy(zv[:, :, :, st:st + sz], src)
                zps = []
                for g in diag:
                    st, sz = g['st'], g['sz']
                    zp = ps_p.tile([128, 1024], F32, tag="ps",
                                   name="zd")
                    zps.append(zp)
                    for cp in range(2):
                        nc.tensor.matmul(
                            zp[:, cp * 512:cp * 512 + 384],
                            cmat(g['gi']),
                            tvs[cp][:, :, st:st + sz])
                for g, zp in zip(diag, zps):
                    st, sz = g['st'], g['sz']
                    src = zp[:].rearrange(
                        "p (c x) -> p c x", c=2)[:, :, 0:384]
                    ev.copy(zv[:, :, :, st:st + sz], src)

            def emit_C(u):
                zsb = zsb_u.pop(u)
                zt = zt_p.tile([128, NSTR_CH * 256], BF16,
                               tag=f"zt{u % 2}", name=f"zt_{u}")
                zt_u[u] = zt
                for jb in range(0, NSTR_CH, 8):
                    zc = ps_p.tile([128, 2048], BF16, tag="ps",
                                   name="zc")
                    for m in range(16):
                        j = jb + m // 2
                        cp = m % 2
                        src = zsb[:, (cp * NSTR_CH + j) * 128:
                                  (cp * NSTR_CH + j + 1) * 128]
                        nc.tensor.transpose(
                            zc[:, m * 128:(m + 1) * 128], src, ident)
                    ev.copy(zt[:, jb * 256:(jb + 8) * 256], zc[:],
                            bf16_src=True)

            def emit_D(u):
                ch, q = divmod(u, NQ)
                zt = zt_u.pop(u)
                for jq in range(0, NSTR_CH, 8):
                    ob = osb_p.tile([128, 2048], BF16, tag="ob")
                    for h in range(2):
                        po = ps_p.tile([128, 1024], F32, tag="ps",
                                       name="po")
                        for m in range(2):
                            nc.tensor.matmul(
                                po[:, m * 512:(m + 1) * 512], LS4,
                                zt[:, (jq + 4 * h + 2 * m) * 256:
                                   (jq + 4 * h + 2 * m + 2) * 256])
                        ev.copy(ob[:, h * 1024:(h + 1) * 1024], po[:])
                    nc.sync.dma_start(out[q, ch * 3 + jq // 8], ob[:])

            # software-pipelined unit loop: B(u) | C(u-1) | D(u-2) so the
            # Z/ZT evacuations hide under the next unit's matmul stream
            for ch in range(NCH):
                t1Tq = [[t1T_p.tile([128, NSTR_CH * 128], BF16,
                                    tag=f"t1T{q}_{cp}",
                                    name=f"t1T_{ch}_{q}_{cp}")
                         for cp in range(2)] for q in range(NQ)]
                emit_A(ch, t1Tq)
                for q in range(NQ):
                    u = ch * NQ + q
                    emit_B(u, t1Tq[q])
                    if u >= 1:
                        emit_C(u - 1)
                    if u >= 2:
                        emit_D(u - 2)
            last = NCH * NQ - 1
            emit_C(last)
            emit_D(last - 1)
            emit_D(last)
    cap_sync_waits(nc)
    return nc


_CACHE = {}


def prepare(W, U_y, U_x, mask, block_rows, block_cols):
    """Compile (cached) and build per-core input maps."""
    # NOTE: --enable-ldw-opt=true rejects the bf16 LDWEIGHTS this kernel
    # emits ("InstLdweights is not compatible with LDW optimization"), and
    # nearly every matmul here has a fresh stationary anyway.
    W = np.ascontiguousarray(np.asarray(W, np.float32))
    cstr, cstb, ginfo, gkey = host_precompute(
        U_y, U_x, mask, block_rows, block_cols)

    key = ("nc_v2", gkey)
    if key not in _CACHE:
        _CACHE[key] = build_kernel(ginfo)
    nc = _CACHE[key]

    in_maps = []
    for core in range(NCORE):
        Wsh = np.ascontiguousarray(W[:, core * CSH:(core + 1) * CSH])
        in_maps.append({"w": Wsh, "cstr": cstr, "cstb": cstb})
    return nc, in_maps


def unshard(results):
    outs = []
    for core in range(NCORE):
        o3 = np.asarray(results[core]["out"]).astype(np.float32)
        o = o3.reshape(3, 2, 3, 128, 8, 256).transpose(
            1, 2, 4, 3, 0, 5).reshape(6144, CSH)
        outs.append(o)
    return np.ascontiguousarray(np.concatenate(outs, axis=1))


def kernel(W, U_y, U_x, mask, block_rows, block_cols):
    from concourse import bass_utils
    nc, in_maps = prepare(W, U_y, U_x, mask, block_rows, block_cols)

    res = None
    last_exc = None
    for attempt in range(3):
        try:
            res = bass_utils.run_bass_kernel_spmd(
                nc, in_maps, core_ids=list(range(NCORE)))
            break
        except Exception as e:  # transient NRT_EXEC_UNIT states recover
            last_exc = e
            time.sleep(20 * (attempt + 1))
    if res is None:
        raise last_exc
    return unshard(res.results)


# revision 18
# speedup vs baseline: 1.1811x; 1.1811x over previous
"""EquivariantProjectorViaSchur — TRN2 Bass kernel (8 NeuronCores, SPMD).

Math (per 64x64 channel block B of W):
    V   = U_y^T B U_x
    P   = A o V + symmetrize(rot 2x2 sub-blocks)
    out = U_y P U_x^T
Pipeline (per core, W shard = 12 c_in block-columns):
    A:  T1T = (U_y^T B)^T per block, via W-stationary f32r matmuls
        (256-wide moving keeps f32r at 1 cyc/col; half the output is
        discarded).  Evac casts to bf16.
    B:  Z[:, o in g] = XG_g @ T1T[:, o in g] (+ sigma terms via +-XJ
        matmuls reading pair-swapped T1T slices) — all bf16, f32 psum.
        The o axis is host-permuted group-major with rot groups split
        [evens | odds] so the sigma matmuls read/write contiguous-ish
        slices and no T1s tensor is ever materialized.
    C:  PE transposes in bf16 (out dtype == in dtype -> bf16 PSUM, so
        the evac runs at the DVE 2x 16-bit rate).
    D:  out = kron(I2, U_y[:,perm]^T)^T @ ZT, bf16 moving, f32 psum,
        evac converts to bf16 for the store.
Evacuations are merged into 2-bank PSUM spans (512-2048 elements per
copy) to amortize the ACT/DVE fixed costs, and split greedily between
the two engines by a cost model.
Sharding: c_in block-columns — core i owns W[:, i*768:(i+1)*768]; the
tiny constant matrices are replicated (precomputed host-side).
"""
import contextlib
import time

import numpy as np
import ml_dtypes

import concourse.bass as bass
import concourse.tile as tile
import concourse.mybir as mybir
from concourse.tile import ScopedClock

F32 = mybir.dt.float32
F32R = mybir.dt.float32r
BF16 = mybir.dt.bfloat16

O = 64
NSTR_CH = 24          # 128-row stripes per b-chunk
NCH = 2               # b chunks of 24 stripes (48 blocks)
NQ = 3                # c quads (4 c-blocks = 256 cols each)
NCORE = 8
CSH = 768             # columns per core shard
NMAT_B = 28           # bf16 const mats: ident, LS4, 10 XG, 8 XJ+, 8 XJ-


# ---------------------------------------------------------------------------
# workarounds for this toolchain
# ---------------------------------------------------------------------------
def _patched_drain_and_barrier(self, tick_clock, wait_clock):
    # this walrus build rejects >1 sem-wait on a Drain: split the tail waits
    drain_inst = self.nc.sync.drain()
    wait_clock.add_sem_waits(drain_inst.ins,
                             ScopedClock({None: tick_clock.global_clock}))
    si = drain_inst.ins.sync_info
    waits = list(si.on_wait) if si is not None else []
    if len(waits) > 1:
        drain_inst.ins.sync_info = mybir.SyncInfo(
            on_wait=waits[:1], on_update=list(si.on_update))
        for i in range(1, len(waits)):
            d2 = self.nc.sync.drain()
            d2.ins.sync_info = mybir.SyncInfo(on_wait=[waits[i]], on_update=[])
    self.nc.all_engine_barrier()
    assert self.sems is not None
    popped = self.nc._tile_sem_poison_stack.pop()
    assert popped is self._sem_poison
    self.nc.clear_and_free_semaphores(list(self.sems.allocated().values()))
    self.nc.all_engine_barrier()


tile.TileContext._drain_and_barrier = _patched_drain_and_barrier


def cap_sync_waits(nc):
    """walrus codegen allows only 1 sem-wait per instruction struct here;
    carry the excess on NoOps inserted just before (same engine/point)."""
    for f in nc.m.functions:
        for blk in f.blocks:
            insts = list(blk.instructions)
            out = []
            ctr = 0
            for ins in insts:
                si = ins.sync_info
                waits = list(si.on_wait) if si is not None else []
                if len(waits) > 1:
                    for i in range(len(waits) - 1):
                        n = mybir.InstNoOp(name=f"{ins.name}_w{ctr}",
                                           ins=[], outs=[])
                        ctr += 1
                        n.engine = ins.engine
                        n.sync_info = mybir.SyncInfo(on_wait=[waits[i]],
                                                     on_update=[])
                        out.append(n)
                    ins.sync_info = mybir.SyncInfo(
                        on_wait=waits[-1:], on_update=list(si.on_update))
                out.append(ins)
            blk.instructions = out


_LDW_PATCHED = False


def enable_ldw_opt():
    """--enable-ldw-opt=true: skip redundant LDWEIGHTS for runs of matmuls
    sharing the same stationary operand."""
    global _LDW_PATCHED
    if _LDW_PATCHED:
        return
    _LDW_PATCHED = True
    import concourse.bass_utils as bu
    orig = bu.bir_verify_and_optimise

    def patched(tmpdir, inp="bir.json", outp="file.neff", arch=None, *,
                dve_root=None):
        real_run = bu.run_command

        def run_hook(argv, **kw):
            argv = ["--enable-ldw-opt=true" if a == "--enable-ldw-opt=false"
                    else a for a in argv]
            return real_run(argv, **kw)
        bu.run_command = run_hook
        try:
            return orig(tmpdir, inp, outp, arch, dve_root=dve_root)
        finally:
            bu.run_command = real_run
    bu.bir_verify_and_optimise = patched


# ---------------------------------------------------------------------------
# host-side precompute of the replicated constants
# ---------------------------------------------------------------------------
def host_precompute(U_y, U_x, mask, block_rows, block_cols):
    rows = np.asarray(block_rows)
    mask = np.asarray(mask)
    U_y64 = np.asarray(U_y, np.float64)
    U_x64 = np.asarray(U_x, np.float64)
    r_rot = set(int(x) for x in rows.tolist())
    nqd = len(rows) // 4
    cols = np.asarray(block_cols)
    for t in range(nqd):
        r = rows[4 * t:4 * t + 4]
        c = cols[4 * t:4 * t + 4]
        assert mask[r, c].all()
        assert r[0] == r[1] and r[2] == r[3] and r[2] == r[0] + 1 \
            and r[0] % 2 == 0
        assert c[0] == c[2] and c[1] == c[3] and c[1] == c[0] + 1 \
            and c[0] % 2 == 0
    groups, seen = [], np.zeros(O, bool)
    for k in range(O):
        if seen[k]:
            continue
        mem = np.where(mask[k] > 0)[0]
        assert (mask[np.ix_(mem, mem)] > 0).all()
        for m in mem:
            seen[m] = True
        groups.append([int(x) for x in mem])
    pi = np.where(np.arange(O) % 2 == 0, 1.0, -1.0)
    eye2 = np.eye(2)

    # o-permutation: rot groups first as [evens | odds], then diag groups
    perm = []
    ginfo = []     # (is_R, slot_start, size, natural members in slot order)
    for mem in groups:
        if int(mem[0]) in r_rot:
            assert len(mem) == (mem[-1] - mem[0] + 1), "rot group gap"
            ev, od = mem[0::2], mem[1::2]
            ginfo.append(dict(is_R=True, st=len(perm), sz=len(mem)))
            perm += list(ev) + list(od)
    for mem in groups:
        if int(mem[0]) not in r_rot:
            st = int(mem[0])
            assert all(int(m) == st + 2 * i for i, m in enumerate(mem)), \
                "diag group not stride-2"
            ginfo.append(dict(is_R=False, st=len(perm), sz=len(mem)))
            perm += list(mem)
    perm = np.asarray(perm)
    assert len(perm) == O

    # f32r const: [kron(I2, U_y[:, perm]) | same] — 256-wide moving for
    # phase A (right half of each product is discarded; >=256 keeps f32r
    # at 1 cyc/col)
    LY = np.kron(eye2, U_y64[:, perm]).astype(np.float32)
    cstr = np.ascontiguousarray(np.concatenate([LY, LY], axis=1))

    # bf16 consts: ident, LS4, XG x10, then per rot group XJ+, XJ-
    matsb = [np.eye(128), np.kron(eye2, U_y64[:, perm].T)]
    for g in ginfo:
        mem = perm[g['st']:g['st'] + g['sz']]
        s = 0.5 if g['is_R'] else 1.0
        a = np.zeros(O)
        a[mem] = 1.0
        XG = s * (U_x64 @ np.diag(a) @ U_x64.T)
        g['gi'] = len(matsb)
        matsb.append(np.kron(eye2, XG))
    for g in ginfo:
        if not g['is_R']:
            continue
        mem = perm[g['st']:g['st'] + g['sz']]
        XJ = np.zeros((O, O))
        for k in mem:
            XJ += 0.5 * pi[k] * np.outer(U_x64[:, k], U_x64[:, k ^ 1])
        g['jp'] = len(matsb)
        matsb.append(np.kron(eye2, XJ.T))
        g['jn'] = len(matsb)
        matsb.append(np.kron(eye2, -XJ.T))
    assert len(matsb) == NMAT_B
    cstb = np.ascontiguousarray(
        np.concatenate(matsb, axis=1).astype(ml_dtypes.bfloat16))
    gkey = tuple((g['is_R'], g['st'], g['sz']) for g in ginfo)
    return cstr, cstb, ginfo, gkey


class _EvacBalancer:
    """Greedy ACT/DVE assignment for PSUM->SBUF copies.

    ACT ACTIVATE: (N + 352)/1.2 ns, dtype-independent.
    DVE from PSUM: fp32 1x -> (N + 151)/0.96; bf16 2x_1P -> (N/2 + 151)/0.96.
    """
    def __init__(self, nc):
        self.nc = nc
        self.t_act = 0.0
        self.t_dve = 0.0

    def copy(self, dst, src, bf16_src=False):
        n = src.free_size()
        c_act = (352.0 + n) / 1.2
        c_dve = (151.0 + (n / 2.0 if bf16_src else float(n))) / 0.96
        if self.t_act + c_act <= self.t_dve + c_dve:
            self.t_act += c_act
            return self.nc.scalar.copy(dst, src)
        else:
            self.t_dve += c_dve
            return self.nc.vector.tensor_copy(dst, src)


# ---------------------------------------------------------------------------
# device kernel (one program, SPMD over 8 cores)
# ---------------------------------------------------------------------------
def build_kernel(ginfo):
    nc = bass.Bass("TRN2", target_bir_lowering=False, debug=False,
                   num_devices=1)
    w = nc.dram_tensor("w", [6144, CSH], F32R, kind="ExternalInput").ap()
    cstr = nc.dram_tensor("cstr", [128, 256], F32R,
                          kind="ExternalInput").ap()
    cstb = nc.dram_tensor("cstb", [128, NMAT_B * 128], BF16,
                          kind="ExternalInput").ap()
    out = nc.dram_tensor("out", [NQ, 12, 128, 1024], BF16,
                         kind="ExternalOutput").ap()

    with tile.TileContext(nc) as tc:
        ctx = contextlib.ExitStack()
        with ctx:
            ev = _EvacBalancer(nc)
            csr_p = ctx.enter_context(tc.tile_pool(name="csr", bufs=1))
            csb_p = ctx.enter_context(tc.tile_pool(name="csb", bufs=1))
            wch_p = ctx.enter_context(tc.tile_pool(name="wch", bufs=17))
            t1T_p = ctx.enter_context(tc.tile_pool(name="t1T", bufs=1))
            z_p = ctx.enter_context(tc.tile_pool(name="z", bufs=2))
            zt_p = ctx.enter_context(tc.tile_pool(name="zt", bufs=2))
            osb_p = ctx.enter_context(tc.tile_pool(name="osb", bufs=8))
            # one shared PSUM pool: 4 bufs x 2 banks = all 8 banks
            ps_p = ctx.enter_context(
                tc.tile_pool(name="ps", bufs=4, space="PSUM"))

            csr = csr_p.tile([128, 256], F32R)
            nc.sync.dma_start(csr[:], cstr[:])
            csb = csb_p.tile([128, NMAT_B * 128], BF16)

            def cmat(i):
                return csb[:, i * 128:(i + 1) * 128]

            ident, LS4 = cmat(0), cmat(1)
            LY2 = csr[:]

            zsb_u = {}
            zt_u = {}

            def emit_A(ch, t1Tq):
                for sg in range(0, NSTR_CH, 4):
                    grp = []
                    for k4 in range(4):
                        s = sg + k4
                        t = wch_p.tile([128, CSH], F32R, tag="w",
                                       name=f"w_{ch}_{s}")
                        r0 = (ch * NSTR_CH + s) * 128
                        nc.sync.dma_start(t[:], w[r0:r0 + 128, :])
                        grp.append(t)
                    if ch == 0 and sg == 0:
                        # bf16 consts aren't needed until phase B; keep
                        # them behind the first W stripes in the queue
                        nc.sync.dma_start(csb[:], cstb[:])
                    for q in range(NQ):
                        for cp in range(2):
                            pa = ps_p.tile([128, 1024], F32, tag="ps",
                                           name="pa")
                            for m in range(4):
                                lhsT = grp[m][:, (q * 2 + cp) * 128:
                                              (q * 2 + cp + 1) * 128]
                                nc.tensor.matmul(
                                    pa[:, m * 256:(m + 1) * 256],
                                    lhsT, LY2)
                            src = pa[:].rearrange(
                                "p (m c) -> p m c", m=4)[:, :, 0:128]
                            ev.copy(
                                t1Tq[q][cp][:, sg * 128:(sg + 4) * 128],
                                src)

            def emit_B(u, t1T):
                tvs = [t1T[cp][:].rearrange("p (b o) -> p b o", o=64)
                       for cp in range(2)]
                zsb = z_p.tile([128, 2 * NSTR_CH * 128], BF16,
                               tag=f"z{u % 2}", name=f"z_{u}")
                zsb_u[u] = zsb
                zv = zsb[:].rearrange("p (c b o) -> p c b o", c=2, o=64)
                # process groups in pairs with two psum tiles in flight so
                # each group's LDW/evac overlaps the partner's matmuls
                rot = [g for g in ginfo if g['is_R']]
                diag = [g for g in ginfo if not g['is_R']]
                pairs = list(zip(rot[0::2], rot[1::2]))
                for ga, gb in pairs:
                    zps = []
                    for g in (ga, gb):
                        st, sz = g['st'], g['sz']
                        zp = ps_p.tile([128, 1024], F32, tag="ps",
                                       name="zp")
                        zps.append(zp)
                        for cp in range(2):
                            nc.tensor.matmul(
                                zp[:, cp * 512:cp * 512 + 288],
                                cmat(g['gi']),
                                tvs[cp][:, :, st:st + sz],
                                start=True, stop=False)
                    for g, zp in zip((ga, gb), zps):
                        st, sz = g['st'], g['sz']
                        h = sz // 2
                        for cp in range(2):
                            zpv = zp[:, cp * 512:cp * 512 + 288] \
                                .rearrange("p (b o) -> p b o", o=6)
                            nc.tensor.matmul(
                                zpv[:, :, 0:h], cmat(g['jp']),
                                tvs[cp][:, :, st + h:st + sz],
                                start=False, stop=False)
                            nc.tensor.matmul(
                                zpv[:, :, h:sz], cmat(g['jn']),
                                tvs[cp][:, :, st:st + h],
                                start=False, stop=True)
                    for g, zp in zip((ga, gb), zps):
                        st, sz = g['st'], g['sz']
                        src = zp[:].rearrange(
                            "p (c x) -> p c x", c=2)[:, :, 0:288]
                        ev.copy(zv[:, :, :, st:st + sz], src)
                zps = []
                for g in diag:
                    st, sz = g['st'], g['sz']
                    zp = ps_p.tile([128, 1024], F32, tag="ps",
                                   name="zd")
                    zps.append(zp)
                    for cp in range(2):
                        nc.tensor.matmul(
                            zp[:, cp * 512:cp * 512 + 384],
                            cmat(g['gi']),
                            tvs[cp][:, :, st:st + sz])
                for g, zp in zip(diag, zps):
                    st, sz = g['st'], g['sz']
                    src = zp[:].rearrange(
                        "p (c x) -> p c x", c=2)[:, :, 0:384]
                    ev.copy(zv[:, :, :, st:st + sz], src)

            def emit_C(u):
                zsb = zsb_u.pop(u)
                zt = zt_p.tile([128, NSTR_CH * 256], BF16,
                               tag=f"zt{u % 2}", name=f"zt_{u}")
                zt_u[u] = zt
                for jb in range(0, NSTR_CH, 8):
                    zc = ps_p.tile([128, 2048], BF16, tag="ps",
                                   name="zc")
                    for m in range(16):
                        j = jb + m // 2
                        cp = m % 2
                        src = zsb[:, (cp * NSTR_CH + j) * 128:
                                  (cp * NSTR_CH + j + 1) * 128]
                        nc.tensor.transpose(
                            zc[:, m * 128:(m + 1) * 128], src, ident)
                    ev.copy(zt[:, jb * 256:(jb + 8) * 256], zc[:],
                            bf16_src=True)

            def emit_D(u):
                ch, q = divmod(u, NQ)
                zt = zt_u.pop(u)
                for jq in range(0, NSTR_CH, 4):
                    po = ps_p.tile([128, 1024], F32, tag="ps",
                                   name="po")
                    for m in range(2):
                        nc.tensor.matmul(
                            po[:, m * 512:(m + 1) * 512], LS4,
                            zt[:, (jq + 2 * m) * 256:
                               (jq + 2 * m + 2) * 256])
                    ob = osb_p.tile([128, 1024], BF16, tag="ob")
                    ev.copy(ob[:], po[:])
                    nc.sync.dma_start(out[q, ch * 6 + jq // 4], ob[:])

            # software-pipelined unit loop: B(u) | C(u-1) | D(u-2) so the
            # Z/ZT evacuations hide under the next unit's matmul stream
            for ch in range(NCH):
                t1Tq = [[t1T_p.tile([128, NSTR_CH * 128], BF16,
                                    tag=f"t1T{q}_{cp}",
                                    name=f"t1T_{ch}_{q}_{cp}")
                         for cp in range(2)] for q in range(NQ)]
                emit_A(ch, t1Tq)
                for q in range(NQ):
                    u = ch * NQ + q
                    emit_B(u, t1Tq[q])
                    if u >= 1:
                        emit_C(u - 1)
                    if u >= 2:
                        emit_D(u - 2)
            last = NCH * NQ - 1
            emit_C(last)
            emit_D(last - 1)
            emit_D(last)
    cap_sync_waits(nc)
    return nc


_CACHE = {}


def prepare(W, U_y, U_x, mask, block_rows, block_cols):
    """Compile (cached) and build per-core input maps."""
    # NOTE: --enable-ldw-opt=true rejects the bf16 LDWEIGHTS this kernel
    # emits ("InstLdweights is not compatible with LDW optimization"), and
    # nearly every matmul here has a fresh stationary anyway.
    W = np.ascontiguousarray(np.asarray(W, np.float32))
    cstr, cstb, ginfo, gkey = host_precompute(
        U_y, U_x, mask, block_rows, block_cols)

    key = ("nc_v2", gkey)
    if key not in _CACHE:
        _CACHE[key] = build_kernel(ginfo)
    nc = _CACHE[key]

    in_maps = []
    for core in range(NCORE):
        Wsh = np.ascontiguousarray(W[:, core * CSH:(core + 1) * CSH])
        in_maps.append({"w": Wsh, "cstr": cstr, "cstb": cstb})
    return nc, in_maps


def unshard(results):
    outs = []
    for core in range(NCORE):
        o3 = np.asarray(results[core]["out"]).astype(np.float32)
        o = o3.reshape(3, 2, 6, 128, 4, 256).transpose(
            1, 2, 4, 3, 0, 5).reshape(6144, CSH)
        outs.append(o)
    return np.ascontiguousarray(np.concatenate(outs, axis=1))


def kernel(W, U_y, U_x, mask, block_rows, block_cols):
    from concourse import bass_utils
    nc, in_maps = prepare(W, U_y, U_x, mask, block_rows, block_cols)

    res = None
    last_exc = None
    for attempt in range(3):
        try:
            res = bass_utils.run_bass_kernel_spmd(
                nc, in_maps, core_ids=list(range(NCORE)))
            break
        except Exception as e:  # transient NRT_EXEC_UNIT states recover
            last_exc = e
            time.sleep(20 * (attempt + 1))
    if res is None:
        raise last_exc
    return unshard(res.results)
